# revision 30
# baseline (speedup 1.0000x reference)
"""Trainium2 Bass kernel for CustomMinkowskiLayerNorm (v6).

Math (matches the jax reference):
    counts[b]  = #points with batch_indices == b           (clamped >= 1)
    mean[b,c]  = sum_{i in b} x[i,c] / counts[b]
    var[b,c]   = sum_{i in b} (x[i,c]-mean)^2 / counts[b]  (= E[x^2]-mean^2)
    out[i,c]   = (x[i,c]-mean[b_i,c]) / sqrt(var[b_i,c]+eps) * gamma[c] + beta[c]

Sharding: batch_indices is sorted and BATCH == n_cores == 8, so each core owns
exactly one batch segment -> all reductions are core-local, no collectives.
Host splits at segment boundaries, transposes each segment to channel-major
and zero-pads:  xt[p, f]: partition p<64 = channel p, points [0, F_HALF);
p>=64 = channel p-64, points [F_HALF, 2*F_HALF).

Design (v6):
  * Whole segment cached in SBUF as fp16 (~15.5 MiB): HBM traffic is 31 MiB
    in + 31 MiB out per core.  fp16 costs ~2e-4 median rel err (tol 2e-2).
  * Loads all on the sync HWDGE ring; stores split across sync + scalar +
    SWDGE rings (measured 410-423 GB/s aggregate for writes).
  * Pass-1 engine balance by WHOLE-TILE ownership (per-op times inflate
    ~15-20% under full DMA load, so convert+stats is ~5.3us/tile of engine
    work; splitting tiles, not ops, balances the two engines at ~80us and
    frees each lpool slot with a single op):
      - B-tiles (even t < nt-2): ScalarE Copy(fp32->fp16, accum_out=sum)
        then Square(fp16, accum_out=sumsq into PSUM scratch).
      - V-tiles (odd t, plus the last): DVE tensor_copy convert then
        bn_stats x4 on the fp16 tile.
  * Aggregation: bn_aggr over most V-groups early (aggr1 -> matmul 1 into
    PSUM); after the last chunk, a short bn_aggr + raw-sum ops merge into
    the B accumulator reduce, and a single final matmul (accumulating onto
    matmul 1) folds partition halves and applies 1/cnt:
        tot[p] = (sums[p%64] + sums[p%64+64]) / cnt.
  * Coefficient tail: negvar = mean^2-E[x^2]; r = ACT Sqrt(-negvar+eps);
    DVE reciprocal; s = gamma*r; t_neg = mean*s-beta; pass 2 = x*s - t_neg.
  * The last tile is loaded as 4x512 chunks so <1us of bn_stats trails the
    final landing.
"""

import os
import sys

for _p in ("/opt/trn_rl_repo", "/root/.axon_site/_ro/trn_rl_repo"):
    if os.path.isdir(_p) and _p not in sys.path:
        sys.path.append(_p)

from contextlib import ExitStack

import numpy as np

import concourse.bacc as bacc
import concourse.tile as tile
from concourse import mybir
from concourse._compat import with_exitstack
from concourse.bass_utils import run_bass_kernel_spmd

F32 = mybir.dt.float32
F16 = mybir.dt.float16

N = 1_000_000
C = 64
BATCH = 8
EPS = 1e-5

P = 128            # SBUF partitions
F_TILE = 2048      # free elems per tile -> [128, 2048] f32 = 1 MiB per DMA
BN_F = 512         # bn_stats free-dim max
LOAD_BUFS = 6      # rotating fp32 load slots
OUT_BUFS = 4       # rotating fp32 pass-2 output slots
SCALAR_LOADS = (3, 7, 11, 15, 19, 23)  # V-tile loads on the scalar ring
SWDGE_STORES = (1, 9)   # tiles stored via the SWDGE ring (3rd write queue)

_mult = mybir.AluOpType.mult
_add = mybir.AluOpType.add
_sub = mybir.AluOpType.subtract
_AF = mybir.ActivationFunctionType


def _tile_plan(nt: int):
    """B-tiles (ScalarE whole-tile stats): even t below nt-2.

    V-tiles (everything else) run on DVE.  agg_split: V-groups of tiles
    below this aggregate early (aggr1); the rest go through the short tail
    aggregation.
    """
    if nt < 8:
        return set(), 0
    b_set = {t for t in range(0, nt - 2, 2)}
    return b_set, nt - 5


def _make_body(f_half: int):
    nt = f_half // F_TILE
    last = nt - 1
    b_set, agg_split_tile = _tile_plan(nt)
    n_b = len(b_set)

    v_tiles = [t for t in range(nt) if t not in b_set]
    grp_of = {}
    g = 0
    ga = 0
    for t in v_tiles:
        grp_of[t] = g
        if t < agg_split_tile:
            ga = g + F_TILE // BN_F
        g += F_TILE // BN_F
    gtot = g
    # elems per partition behind aggr1 / tail-aggr (compile-time constants)
    n1 = sum(F_TILE for t in v_tiles if t < agg_split_tile)
    n2 = sum(F_TILE for t in v_tiles if t >= agg_split_tile)

    @with_exitstack
    def _body(ctx: ExitStack, tc: tile.TileContext,
              out_ap, xt_ap, gcol_ap, bcol_ap, fold1_ap, fold3_ap):
        nc = tc.nc
        ngrp = F_TILE // BN_F

        cache = ctx.enter_context(tc.tile_pool(name="cache", bufs=nt))
        lpool = ctx.enter_context(tc.tile_pool(name="lpool", bufs=LOAD_BUFS))
        opool = ctx.enter_context(tc.tile_pool(name="opool", bufs=OUT_BUFS))
        small = ctx.enter_context(tc.tile_pool(name="small", bufs=1))
        psum = ctx.enter_context(tc.tile_pool(name="psum", bufs=1, space="PSUM"))

        stats = small.tile([P, max(gtot, 1), 6], F32, tag="stats")
        accs = None
        psq = None
        if n_b:
            accs = small.tile([P, n_b, 2], F32, tag="accs")
            psq = psum.tile([P, F_TILE], F32, tag="psq")

        # Small inputs ride the SWDGE ring; they land well before first use
        # and never delay the HWDGE load burst.
        gcol_sb = small.tile([P, 1], F32, tag="gcol")
        bcol_sb = small.tile([P, 1], F32, tag="bcol")
        fold1_sb = small.tile([P, P], F32, tag="fold1")
        fold3_sb = small.tile([P, P], F32, tag="fold3")
        nc.gpsimd.dma_start(out=fold1_sb, in_=fold1_ap)
        nc.gpsimd.dma_start(out=fold3_sb, in_=fold3_ap)
        nc.gpsimd.dma_start(out=gcol_sb, in_=gcol_ap)
        nc.gpsimd.dma_start(out=bcol_sb, in_=bcol_ap)

        # Pre-load the ACT sqrt table so the tail doesn't pay ACT_TABLE_LOAD;
        # eps lives in a tiny tile (no const AP registered for 1e-5).
        warm = small.tile([P, 1], F32, tag="warm")
        nc.vector.memset(warm, 1.0)
        eps_sb = small.tile([P, 1], F32, tag="eps")
        nc.vector.memset(eps_sb, EPS)
        nc.scalar.activation(out=warm, in_=warm, func=_AF.Sqrt)

        ptot = psum.tile([P, 2], F32, tag="ptot")

        # ---- pass 1: single load stream on sync; B-tiles on ScalarE,
        # V-tiles on DVE ----
        cached = {}
        b_idx = {t: i for i, t in enumerate(sorted(b_set))}
        mva = small.tile([P, 2], F32, tag="mva")
        mvb = small.tile([P, 2], F32, tag="mvb")
        cols1 = small.tile([P, 2], F32, tag="cols1")
        mm1_emitted = False

        for t in range(nt):
            sl = slice(t * F_TILE, (t + 1) * F_TILE)
            xt16 = cache.tile([P, F_TILE], F16, tag="c")
            cached[t] = xt16
            xt32 = lpool.tile([P, F_TILE], F32, tag="l")
            if t == last:
                # Final tile in 4 chunks: DVE convert+stats pipeline with
                # the chunk DMAs; <1us of work follows the last landing.
                for j in range(ngrp):
                    cs = slice(t * F_TILE + j * BN_F, t * F_TILE + (j + 1) * BN_F)
                    nc.sync.dma_start(out=xt32[:, j * BN_F:(j + 1) * BN_F],
                                      in_=xt_ap[:, cs])
                for j in range(ngrp):
                    c32 = xt32[:, j * BN_F:(j + 1) * BN_F]
                    c16 = xt16[:, j * BN_F:(j + 1) * BN_F]
                    nc.vector.bn_stats(out=stats[:, grp_of[t] + j, :], in_=c32)
                    nc.vector.tensor_copy(out=c16, in_=c32)
            else:
                eng = nc.scalar if t in SCALAR_LOADS else nc.sync
                eng.dma_start(out=xt32, in_=xt_ap[:, sl])
                if t in b_set:
                    bi = b_idx[t]
                    nc.scalar.activation(out=xt16, in_=xt32, func=_AF.Copy,
                                         accum_out=accs[:, bi, 0:1])
                    nc.scalar.activation(out=psq, in_=xt32, func=_AF.Square,
                                         accum_out=accs[:, bi, 1:2])
                else:
                    nc.vector.tensor_copy(out=xt16, in_=xt32)
                    for j in range(ngrp):
                        nc.vector.bn_stats(
                            out=stats[:, grp_of[t] + j, :],
                            in_=xt32[:, j * BN_F:(j + 1) * BN_F])
            if t == agg_split_tile - 1 and ga > 0:
                # Early aggregation of V-groups so far -> first fold matmul
                # (runs on DVE/PE while the tail tiles stream in).
                nc.vector.bn_aggr(out=mva, in_=stats[:, :ga, :])
                nc.vector.tensor_mul(out=cols1[:, 1:2], in0=mva[:, 0:1],
                                     in1=mva[:, 0:1])
                nc.vector.tensor_add(out=cols1[:, 1:2], in0=cols1[:, 1:2],
                                     in1=mva[:, 1:2])
                nc.vector.tensor_scalar_mul(out=cols1[:, 1:2],
                                            in0=cols1[:, 1:2],
                                            scalar1=float(n1))
                nc.vector.tensor_scalar_mul(out=cols1[:, 0:1],
                                            in0=mva[:, 0:1],
                                            scalar1=float(n1))
                nc.tensor.matmul(out=ptot, lhsT=fold1_sb, rhs=cols1,
                                 start=True, stop=False)
                mm1_emitted = True

        # ---- tail aggregation: short bn_aggr + raw sums, merged with the
        # B accumulators into ONE final fold matmul ----
        asums = small.tile([P, 2], F32, tag="asums")
        if n_b:
            acc_view = accs.rearrange("p t c -> p c t")
            nc.vector.reduce_sum(out=asums, in_=acc_view,
                                 axis=mybir.AxisListType.X)
        else:
            nc.vector.memset(asums, 0.0)
        if ga > 0:
            nc.vector.bn_aggr(out=mvb, in_=stats[:, ga:, :])
        else:
            nc.vector.bn_aggr(out=mvb, in_=stats[:, :, :])
        cols2 = small.tile([P, 2], F32, tag="cols2")
        nc.vector.tensor_mul(out=cols2[:, 1:2], in0=mvb[:, 0:1], in1=mvb[:, 0:1])
        nc.vector.tensor_add(out=cols2[:, 1:2], in0=cols2[:, 1:2],
                             in1=mvb[:, 1:2])
        nc.vector.tensor_scalar_mul(out=cols2[:, 1:2], in0=cols2[:, 1:2],
                                    scalar1=float(n2 if ga > 0 else n1 + n2))
        nc.vector.tensor_scalar_mul(out=cols2[:, 0:1], in0=mvb[:, 0:1],
                                    scalar1=float(n2 if ga > 0 else n1 + n2))
        nc.vector.tensor_add(out=asums, in0=asums, in1=cols2)
        nc.tensor.matmul(out=ptot, lhsT=fold3_sb, rhs=asums,
                         start=not mm1_emitted, stop=True)

        # ---- per-channel coefficients ----
        tot = small.tile([P, 2], F32, tag="tot")   # (mean, E[x^2]) per channel
        nc.vector.tensor_copy(out=tot, in_=ptot)
        negv = small.tile([P, 1], F32, tag="negv")  # mean^2 - E[x^2]
        nc.vector.tensor_scalar(out=negv, in0=tot[:, 0:1],
                                scalar1=tot[:, 0:1], scalar2=tot[:, 1:2],
                                op0=_mult, op1=_sub)
        r = small.tile([P, 1], F32, tag="r")        # sqrt(var+eps) -> 1/...
        nc.scalar.activation(out=r, in_=negv, func=_AF.Sqrt,
                             scale=-1.0, bias=eps_sb[:, 0:1])
        nc.vector.reciprocal(out=r, in_=r)
        s_col = small.tile([P, 1], F32, tag="s_col")
        nc.vector.tensor_mul(out=s_col, in0=gcol_sb, in1=r)
        tneg = small.tile([P, 1], F32, tag="tneg")  # mean*s - beta
        nc.vector.tensor_scalar(out=tneg, in0=tot[:, 0:1],
                                scalar1=s_col[:, 0:1], scalar2=bcol_sb[:, 0:1],
                                op0=_mult, op1=_sub)

        # ---- pass 2: out32 = cache16*s - tneg; stores on all three rings.
        # Tile 0 goes out in halves so the first store issues ~0.8us sooner
        # and both HWDGE rings ramp together. ----
        half = F_TILE // 2
        o32 = opool.tile([P, F_TILE], F32, tag="o")
        for h in range(2):
            hs = slice(h * half, (h + 1) * half)
            nc.vector.tensor_scalar(out=o32[:, hs], in0=cached[0][:, hs],
                                    scalar1=s_col[:, 0:1], scalar2=tneg[:, 0:1],
                                    op0=_mult, op1=_sub)
            eng = nc.sync if h == 0 else nc.scalar
            eng.dma_start(out=out_ap[:, hs], in_=o32[:, hs])
        for i in range(1, nt):
            sl = slice(i * F_TILE, (i + 1) * F_TILE)
            o32 = opool.tile([P, F_TILE], F32, tag="o")
            nc.vector.tensor_scalar(out=o32, in0=cached[i],
                                    scalar1=s_col[:, 0:1], scalar2=tneg[:, 0:1],
                                    op0=_mult, op1=_sub)
            if i in SWDGE_STORES:
                nc.gpsimd.dma_start(out=out_ap[:, sl], in_=o32)
            elif i % 2 == 0:
                nc.sync.dma_start(out=out_ap[:, sl], in_=o32)
            else:
                nc.scalar.dma_start(out=out_ap[:, sl], in_=o32)

    return _body


_NC_CACHE = {}


def _build_program(f_half: int):
    if f_half in _NC_CACHE:
        return _NC_CACHE[f_half]
    nc = bacc.Bacc("TRN2", target_bir_lowering=False, debug=False,
                   num_devices=BATCH)
    xt = nc.dram_tensor("xt", [P, f_half], F32, kind="ExternalInput").ap()
    gcol = nc.dram_tensor("gcol", [P, 1], F32, kind="ExternalInput").ap()
    bcol = nc.dram_tensor("bcol", [P, 1], F32, kind="ExternalInput").ap()
    fold1 = nc.dram_tensor("fold1", [P, P], F32, kind="ExternalInput").ap()
    fold3 = nc.dram_tensor("fold3", [P, P], F32, kind="ExternalInput").ap()
    out = nc.dram_tensor("out", [P, f_half], F32, kind="ExternalOutput").ap()
    with tile.TileContext(nc) as tc:
        _make_body(f_half)(tc, out, xt, gcol, bcol, fold1, fold3)
    nc.compile()
    _NC_CACHE[f_half] = nc
    return nc


def _prepare(features, batch_indices, gamma, beta):
    features = np.asarray(features, dtype=np.float32)
    batch_indices = np.asarray(batch_indices, dtype=np.int32)
    gamma = np.asarray(gamma, dtype=np.float32)
    beta = np.asarray(beta, dtype=np.float32)

    bounds = np.searchsorted(batch_indices, np.arange(BATCH + 1), side="left")
    cnts = np.diff(bounds)
    # fixed SPMD shape: half-row length, padded to a multiple of F_TILE
    f_half = max(int(-(-int(cnts.max()) // 2 // F_TILE) * F_TILE), F_TILE)

    gcol = np.concatenate([gamma, gamma]).reshape(P, 1).astype(np.float32)
    bcol = np.concatenate([beta, beta]).reshape(P, 1).astype(np.float32)
    k = np.arange(P)
    foldm = (k[:, None] % C == k[None, :] % C).astype(np.float32)

    in_maps = []
    for b in range(BATCH):
        s, e = int(bounds[b]), int(bounds[b + 1])
        cnt = e - s
        xt = np.zeros((P, f_half), dtype=np.float32)
        npts1 = min(cnt, f_half)
        if npts1 > 0:
            xt[0:C, :npts1] = features[s : s + npts1].T
        if cnt > f_half:
            xt[C:P, : cnt - f_half] = features[s + f_half : e].T
        inv = 1.0 / max(cnt, 1)
        in_maps.append({
            "xt": xt,
            "gcol": gcol,
            "bcol": bcol,
            "fold1": (foldm * inv).astype(np.float32),
            "fold3": (foldm * inv).astype(np.float32),
        })
    return in_maps, bounds, f_half


def _assemble(results, bounds, f_half):
    out = np.empty((N, C), dtype=np.float32)
    for b in range(BATCH):
        s, e = int(bounds[b]), int(bounds[b + 1])
        cnt = e - s
        if cnt == 0:
            continue
        ot = results[b]["out"]
        npts1 = min(cnt, f_half)
        out[s : s + npts1] = ot[0:C, :npts1].T
        if cnt > f_half:
            out[s + f_half : e] = ot[C:P, : cnt - f_half].T
    return out


def run_with_results(features, batch_indices, gamma, beta, **run_kwargs):
    in_maps, bounds, f_half = _prepare(features, batch_indices, gamma, beta)
    nc = _build_program(f_half)
    res = run_bass_kernel_spmd(nc, in_maps, core_ids=list(range(BATCH)),
                               **run_kwargs)
    return _assemble(res.results, bounds, f_half), res


def kernel(features, batch_indices, gamma, beta):
    out, _ = run_with_results(features, batch_indices, gamma, beta)
    return out


# revision 33
# speedup vs baseline: 1.0714x; 1.0714x over previous
"""Trainium2 Bass kernel for CustomMinkowskiLayerNorm.

Math (matches the jax reference):
    counts[b]  = #points with batch_indices == b           (clamped >= 1)
    mean[b,c]  = sum_{i in b} x[i,c] / counts[b]
    var[b,c]   = sum_{i in b} (x[i,c]-mean)^2 / counts[b]  (= E[x^2]-mean^2)
    out[i,c]   = (x[i,c]-mean[b_i,c]) / sqrt(var[b_i,c]+eps) * gamma[c] + beta[c]

Sharding: batch_indices is sorted and BATCH == n_cores == 8, so each core owns
exactly one batch segment -> all reductions are core-local, no collectives.
Host splits at segment boundaries, transposes each segment to channel-major
and zero-pads:  xt[p, f]: partition p<64 = channel p, points [0, F_HALF);
p>=64 = channel p-64, points [F_HALF, 2*F_HALF).

Design:
  * Whole segment cached in SBUF as fp16 (~15.5 MiB): HBM traffic is 31 MiB
    in + 31 MiB out per core.  fp16 costs ~2e-4 median rel err (tol 2e-2).
  * Loads mostly on the sync HWDGE ring (a few V-tiles on the scalar ring);
    stores split across sync + scalar + SWDGE rings (writes measured at
    400-425 GB/s aggregate on three queues vs ~300 on one).
  * Pass-1 engine balance by WHOLE-TILE ownership (balances the two engines
    and keeps the lpool slot-recycle chain on a single engine per tile):
      - B-tiles (even t < nt-2): ScalarE Copy(fp32->fp16, accum_out=sum)
        then Square(fp32, accum_out=sumsq into PSUM scratch).
      - V-tiles (odd t, plus the last): DVE tensor_copy convert then
        bn_stats x4 on the fp32 source tile.
  * Aggregation: bn_aggr over most V-groups early (aggr1 -> matmul 1 into
    PSUM); after the last chunk, a short bn_aggr + raw-sum ops merge into
    the B accumulator reduce, and a single final matmul (accumulating onto
    matmul 1) folds partition halves and applies 1/cnt:
        tot[p] = (sums[p%64] + sums[p%64+64]) / cnt.
  * Coefficient tail: negvar = mean^2-E[x^2]; r = ACT Sqrt(-negvar+eps);
    DVE reciprocal; s = gamma*r; t_neg = mean*s-beta; pass 2 = x*s - t_neg.
  * The last tile is loaded as 4x512 chunks so <1us of bn_stats trails the
    final landing.
"""

import os
import sys

for _p in ("/opt/trn_rl_repo", "/root/.axon_site/_ro/trn_rl_repo"):
    if os.path.isdir(_p) and _p not in sys.path:
        sys.path.append(_p)

from contextlib import ExitStack

import numpy as np

import concourse.bacc as bacc
import concourse.tile as tile
from concourse import mybir
from concourse._compat import with_exitstack
from concourse.bass_utils import run_bass_kernel_spmd

F32 = mybir.dt.float32
F16 = mybir.dt.float16

N = 1_000_000
C = 64
BATCH = 8
EPS = 1e-5

P = 128            # SBUF partitions
F_TILE = 2048      # free elems per tile -> [128, 2048] f32 = 1 MiB per DMA
BN_F = 512         # bn_stats free-dim max
LOAD_BUFS = 4      # rotating fp32 load slots
OUT_BUFS = 5       # rotating fp32 pass-2 output slots
SCALAR_LOADS = (3, 7, 11, 15, 19, 23)  # V-tile loads on the scalar ring
SWDGE_STORES = (1, 9)   # tiles stored via the SWDGE ring (3rd write queue)

_mult = mybir.AluOpType.mult
_add = mybir.AluOpType.add
_sub = mybir.AluOpType.subtract
_AF = mybir.ActivationFunctionType


def _tile_plan(nt: int):
    """B-tiles (ScalarE whole-tile stats): even t below nt-2.

    V-tiles (everything else) run on DVE.  agg_split: V-groups of tiles
    below this aggregate early (aggr1); the rest go through the short tail
    aggregation.
    """
    if nt < 8:
        return set(), 0
    b_set = {t for t in range(0, nt - 2, 2)}
    return b_set, nt - 5


def _make_body(f_half: int):
    nt = f_half // F_TILE
    last = nt - 1
    b_set, agg_split_tile = _tile_plan(nt)
    n_b = len(b_set)

    v_tiles = [t for t in range(nt) if t not in b_set]
    grp_of = {}
    g = 0
    ga = 0
    for t in v_tiles:
        grp_of[t] = g
        if t < agg_split_tile:
            ga = g + F_TILE // BN_F
        g += F_TILE // BN_F
    gtot = g
    # elems per partition behind aggr1 / tail-aggr (compile-time constants)
    n1 = sum(F_TILE for t in v_tiles if t < agg_split_tile)
    n2 = sum(F_TILE for t in v_tiles if t >= agg_split_tile)

    @with_exitstack
    def _body(ctx: ExitStack, tc: tile.TileContext,
              out_ap, xt_ap, gcol_ap, bcol_ap, fold1_ap, fold3_ap):
        nc = tc.nc
        ngrp = F_TILE // BN_F

        cache = ctx.enter_context(tc.tile_pool(name="cache", bufs=nt))
        lpool = ctx.enter_context(tc.tile_pool(name="lpool", bufs=LOAD_BUFS))
        opool = ctx.enter_context(tc.tile_pool(name="opool", bufs=OUT_BUFS))
        small = ctx.enter_context(tc.tile_pool(name="small", bufs=1))
        psum = ctx.enter_context(tc.tile_pool(name="psum", bufs=1, space="PSUM"))

        stats = small.tile([P, max(gtot, 1), 6], F32, tag="stats")
        accs = None
        psq = None
        if n_b:
            accs = small.tile([P, n_b, 2], F32, tag="accs")
            psq = psum.tile([P, F_TILE], F32, tag="psq")

        # Small inputs ride the SWDGE ring; they land well before first use
        # and never delay the HWDGE load burst.
        gcol_sb = small.tile([P, 1], F32, tag="gcol")
        bcol_sb = small.tile([P, 1], F32, tag="bcol")
        fold1_sb = small.tile([P, P], F32, tag="fold1")
        fold3_sb = small.tile([P, P], F32, tag="fold3")
        nc.gpsimd.dma_start(out=fold1_sb, in_=fold1_ap)
        nc.gpsimd.dma_start(out=fold3_sb, in_=fold3_ap)
        nc.gpsimd.dma_start(out=gcol_sb, in_=gcol_ap)
        nc.gpsimd.dma_start(out=bcol_sb, in_=bcol_ap)

        # Pre-load the ACT sqrt table so the tail doesn't pay ACT_TABLE_LOAD;
        # eps lives in a tiny tile (no const AP registered for 1e-5).
        warm = small.tile([P, 1], F32, tag="warm")
        nc.vector.memset(warm, 1.0)
        eps_sb = small.tile([P, 1], F32, tag="eps")
        nc.vector.memset(eps_sb, EPS)
        nc.scalar.activation(out=warm, in_=warm, func=_AF.Sqrt)

        ptot = psum.tile([P, 2], F32, tag="ptot")

        # ---- pass 1: single load stream on sync; B-tiles on ScalarE,
        # V-tiles on DVE ----
        cached = {}
        b_idx = {t: i for i, t in enumerate(sorted(b_set))}
        mva = small.tile([P, 2], F32, tag="mva")
        mvb = small.tile([P, 2], F32, tag="mvb")
        cols1 = small.tile([P, 2], F32, tag="cols1")
        mm1_emitted = False

        for t in range(nt):
            sl = slice(t * F_TILE, (t + 1) * F_TILE)
            xt16 = cache.tile([P, F_TILE], F16, tag="c")
            cached[t] = xt16
            xt32 = lpool.tile([P, F_TILE], F32, tag="l")
            if t == last:
                # Final tile in 4 chunks: DVE convert+stats pipeline with
                # the chunk DMAs; <1us of work follows the last landing.
                for j in range(ngrp):
                    cs = slice(t * F_TILE + j * BN_F, t * F_TILE + (j + 1) * BN_F)
                    nc.sync.dma_start(out=xt32[:, j * BN_F:(j + 1) * BN_F],
                                      in_=xt_ap[:, cs])
                for j in range(ngrp):
                    c32 = xt32[:, j * BN_F:(j + 1) * BN_F]
                    c16 = xt16[:, j * BN_F:(j + 1) * BN_F]
                    nc.vector.bn_stats(out=stats[:, grp_of[t] + j, :], in_=c32)
                    nc.vector.tensor_copy(out=c16, in_=c32)
            else:
                eng = nc.scalar if t in SCALAR_LOADS else nc.sync
                eng.dma_start(out=xt32, in_=xt_ap[:, sl])
                if t in b_set:
                    bi = b_idx[t]
                    nc.scalar.activation(out=xt16, in_=xt32, func=_AF.Copy,
                                         accum_out=accs[:, bi, 0:1])
                    nc.scalar.activation(out=psq, in_=xt32, func=_AF.Square,
                                         accum_out=accs[:, bi, 1:2])
                else:
                    nc.vector.tensor_copy(out=xt16, in_=xt32)
                    for j in range(ngrp):
                        nc.vector.bn_stats(
                            out=stats[:, grp_of[t] + j, :],
                            in_=xt32[:, j * BN_F:(j + 1) * BN_F])
            if t == agg_split_tile - 1 and ga > 0:
                # Early aggregation of V-groups so far -> first fold matmul
                # (runs on DVE/PE while the tail tiles stream in).
                nc.vector.bn_aggr(out=mva, in_=stats[:, :ga, :])
                nc.vector.tensor_mul(out=cols1[:, 1:2], in0=mva[:, 0:1],
                                     in1=mva[:, 0:1])
                nc.vector.tensor_add(out=cols1[:, 1:2], in0=cols1[:, 1:2],
                                     in1=mva[:, 1:2])
                nc.vector.tensor_scalar_mul(out=cols1[:, 1:2],
                                            in0=cols1[:, 1:2],
                                            scalar1=float(n1))
                nc.vector.tensor_scalar_mul(out=cols1[:, 0:1],
                                            in0=mva[:, 0:1],
                                            scalar1=float(n1))
                nc.tensor.matmul(out=ptot, lhsT=fold1_sb, rhs=cols1,
                                 start=True, stop=False)
                mm1_emitted = True

        # ---- tail aggregation: short bn_aggr + raw sums, merged with the
        # B accumulators into ONE final fold matmul ----
        asums = small.tile([P, 2], F32, tag="asums")
        if n_b:
            acc_view = accs.rearrange("p t c -> p c t")
            nc.vector.reduce_sum(out=asums, in_=acc_view,
                                 axis=mybir.AxisListType.X)
        else:
            nc.vector.memset(asums, 0.0)
        if ga > 0:
            nc.vector.bn_aggr(out=mvb, in_=stats[:, ga:, :])
        else:
            nc.vector.bn_aggr(out=mvb, in_=stats[:, :, :])
        cols2 = small.tile([P, 2], F32, tag="cols2")
        nc.vector.tensor_mul(out=cols2[:, 1:2], in0=mvb[:, 0:1], in1=mvb[:, 0:1])
        nc.vector.tensor_add(out=cols2[:, 1:2], in0=cols2[:, 1:2],
                             in1=mvb[:, 1:2])
        nc.vector.tensor_scalar_mul(out=cols2[:, 1:2], in0=cols2[:, 1:2],
                                    scalar1=float(n2 if ga > 0 else n1 + n2))
        nc.vector.tensor_scalar_mul(out=cols2[:, 0:1], in0=mvb[:, 0:1],
                                    scalar1=float(n2 if ga > 0 else n1 + n2))
        nc.vector.tensor_add(out=asums, in0=asums, in1=cols2)
        nc.tensor.matmul(out=ptot, lhsT=fold3_sb, rhs=asums,
                         start=not mm1_emitted, stop=True)

        # ---- per-channel coefficients ----
        tot = small.tile([P, 2], F32, tag="tot")   # (mean, E[x^2]) per channel
        nc.vector.tensor_copy(out=tot, in_=ptot)
        negv = small.tile([P, 1], F32, tag="negv")  # mean^2 - E[x^2]
        nc.vector.tensor_scalar(out=negv, in0=tot[:, 0:1],
                                scalar1=tot[:, 0:1], scalar2=tot[:, 1:2],
                                op0=_mult, op1=_sub)
        r = small.tile([P, 1], F32, tag="r")        # sqrt(var+eps) -> 1/...
        nc.scalar.activation(out=r, in_=negv, func=_AF.Sqrt,
                             scale=-1.0, bias=eps_sb[:, 0:1])
        nc.vector.reciprocal(out=r, in_=r)
        s_col = small.tile([P, 1], F32, tag="s_col")
        nc.vector.tensor_mul(out=s_col, in0=gcol_sb, in1=r)
        tneg = small.tile([P, 1], F32, tag="tneg")  # mean*s - beta
        nc.vector.tensor_scalar(out=tneg, in0=tot[:, 0:1],
                                scalar1=s_col[:, 0:1], scalar2=bcol_sb[:, 0:1],
                                op0=_mult, op1=_sub)

        # ---- pass 2: out32 = cache16*s - tneg; stores on all three rings ----
        for i in range(nt):
            sl = slice(i * F_TILE, (i + 1) * F_TILE)
            o32 = opool.tile([P, F_TILE], F32, tag="o")
            nc.vector.tensor_scalar(out=o32, in0=cached[i],
                                    scalar1=s_col[:, 0:1], scalar2=tneg[:, 0:1],
                                    op0=_mult, op1=_sub)
            if i in SWDGE_STORES:
                nc.gpsimd.dma_start(out=out_ap[:, sl], in_=o32)
            elif i % 2 == 0:
                nc.sync.dma_start(out=out_ap[:, sl], in_=o32)
            else:
                nc.scalar.dma_start(out=out_ap[:, sl], in_=o32)

    return _body


_NC_CACHE = {}


def _build_program(f_half: int):
    if f_half in _NC_CACHE:
        return _NC_CACHE[f_half]
    nc = bacc.Bacc("TRN2", target_bir_lowering=False, debug=False,
                   num_devices=BATCH)
    xt = nc.dram_tensor("xt", [P, f_half], F32, kind="ExternalInput").ap()
    gcol = nc.dram_tensor("gcol", [P, 1], F32, kind="ExternalInput").ap()
    bcol = nc.dram_tensor("bcol", [P, 1], F32, kind="ExternalInput").ap()
    fold1 = nc.dram_tensor("fold1", [P, P], F32, kind="ExternalInput").ap()
    fold3 = nc.dram_tensor("fold3", [P, P], F32, kind="ExternalInput").ap()
    out = nc.dram_tensor("out", [P, f_half], F32, kind="ExternalOutput").ap()
    with tile.TileContext(nc) as tc:
        _make_body(f_half)(tc, out, xt, gcol, bcol, fold1, fold3)
    nc.compile()
    _NC_CACHE[f_half] = nc
    return nc


def _prepare(features, batch_indices, gamma, beta):
    features = np.asarray(features, dtype=np.float32)
    batch_indices = np.asarray(batch_indices, dtype=np.int32)
    gamma = np.asarray(gamma, dtype=np.float32)
    beta = np.asarray(beta, dtype=np.float32)

    bounds = np.searchsorted(batch_indices, np.arange(BATCH + 1), side="left")
    cnts = np.diff(bounds)
    # fixed SPMD shape: half-row length, padded to a multiple of F_TILE
    f_half = max(int(-(-int(cnts.max()) // 2 // F_TILE) * F_TILE), F_TILE)

    gcol = np.concatenate([gamma, gamma]).reshape(P, 1).astype(np.float32)
    bcol = np.concatenate([beta, beta]).reshape(P, 1).astype(np.float32)
    k = np.arange(P)
    foldm = (k[:, None] % C == k[None, :] % C).astype(np.float32)

    in_maps = []
    for b in range(BATCH):
        s, e = int(bounds[b]), int(bounds[b + 1])
        cnt = e - s
        xt = np.zeros((P, f_half), dtype=np.float32)
        npts1 = min(cnt, f_half)
        if npts1 > 0:
            xt[0:C, :npts1] = features[s : s + npts1].T
        if cnt > f_half:
            xt[C:P, : cnt - f_half] = features[s + f_half : e].T
        inv = 1.0 / max(cnt, 1)
        in_maps.append({
            "xt": xt,
            "gcol": gcol,
            "bcol": bcol,
            "fold1": (foldm * inv).astype(np.float32),
            "fold3": (foldm * inv).astype(np.float32),
        })
    return in_maps, bounds, f_half


def _assemble(results, bounds, f_half):
    out = np.empty((N, C), dtype=np.float32)
    for b in range(BATCH):
        s, e = int(bounds[b]), int(bounds[b + 1])
        cnt = e - s
        if cnt == 0:
            continue
        ot = results[b]["out"]
        npts1 = min(cnt, f_half)
        out[s : s + npts1] = ot[0:C, :npts1].T
        if cnt > f_half:
            out[s + f_half : e] = ot[C:P, : cnt - f_half].T
    return out


def run_with_results(features, batch_indices, gamma, beta, **run_kwargs):
    in_maps, bounds, f_half = _prepare(features, batch_indices, gamma, beta)
    nc = _build_program(f_half)
    res = run_bass_kernel_spmd(nc, in_maps, core_ids=list(range(BATCH)),
                               **run_kwargs)
    return _assemble(res.results, bounds, f_half), res


def kernel(features, batch_indices, gamma, beta):
    out, _ = run_with_results(features, batch_indices, gamma, beta)
    return out


# revision 35
# speedup vs baseline: 1.1403x; 1.0644x over previous
"""Trainium2 Bass kernel for CustomMinkowskiLayerNorm.

Math (matches the jax reference):
    counts[b]  = #points with batch_indices == b           (clamped >= 1)
    mean[b,c]  = sum_{i in b} x[i,c] / counts[b]
    var[b,c]   = sum_{i in b} (x[i,c]-mean)^2 / counts[b]  (= E[x^2]-mean^2)
    out[i,c]   = (x[i,c]-mean[b_i,c]) / sqrt(var[b_i,c]+eps) * gamma[c] + beta[c]

Sharding: batch_indices is sorted and BATCH == n_cores == 8, so each core owns
exactly one batch segment -> all reductions are core-local, no collectives.
Host splits at segment boundaries, transposes each segment to channel-major
and zero-pads:  xt[p, f]: partition p<64 = channel p, points [0, F_HALF);
p>=64 = channel p-64, points [F_HALF, 2*F_HALF).

Design:
  * Whole segment cached in SBUF as fp16 (~15.5 MiB): HBM traffic is 31 MiB
    in + 31 MiB out per core.  fp16 costs ~2e-4 median rel err (tol 2e-2).
  * Loads mostly on the sync HWDGE ring (a few V-tiles on the scalar ring);
    stores split across sync + scalar + SWDGE rings (writes measured at
    400-425 GB/s aggregate on three queues vs ~300 on one).
  * Pass-1 engine balance by WHOLE-TILE ownership (balances the two engines
    and keeps the lpool slot-recycle chain on a single engine per tile):
      - B-tiles (even t < nt-2): ScalarE Copy(fp32->fp16, accum_out=sum)
        then Square(fp32, accum_out=sumsq into PSUM scratch).
      - V-tiles (odd t, plus the last): DVE tensor_copy convert then
        bn_stats x4 on the fp32 source tile.
  * Aggregation: bn_aggr over most V-groups early (aggr1 -> matmul 1 into
    PSUM); after the last chunk, a short bn_aggr + raw-sum ops merge into
    the B accumulator reduce, and a single final matmul (accumulating onto
    matmul 1) folds partition halves and applies 1/cnt:
        tot[p] = (sums[p%64] + sums[p%64+64]) / cnt.
  * Coefficient tail: negvar = mean^2-E[x^2]; r = ACT Sqrt(-negvar+eps);
    DVE reciprocal; s = gamma*r; t_neg = mean*s-beta; pass 2 = x*s - t_neg.
  * The last tile is loaded as 4x512 chunks so <1us of bn_stats trails the
    final landing.
"""

import os
import sys

for _p in ("/opt/trn_rl_repo", "/root/.axon_site/_ro/trn_rl_repo"):
    if os.path.isdir(_p) and _p not in sys.path:
        sys.path.append(_p)

from contextlib import ExitStack

import numpy as np

import concourse.bacc as bacc
import concourse.tile as tile
from concourse import mybir
from concourse._compat import with_exitstack
from concourse.bass_utils import run_bass_kernel_spmd

F32 = mybir.dt.float32
F16 = mybir.dt.float16

N = 1_000_000
C = 64
BATCH = 8
EPS = 1e-5

P = 128            # SBUF partitions
F_TILE = 2048      # free elems per tile -> [128, 2048] f32 = 1 MiB per DMA
BN_F = 512         # bn_stats free-dim max
LOAD_BUFS = 4      # rotating fp32 load slots
OUT_BUFS = 5       # rotating fp32 pass-2 output slots
SCALAR_LOADS = (3, 7, 11, 15, 19, 23)  # V-tile loads on the scalar ring
SWDGE_STORES = (1, 9)   # tiles stored via the SWDGE ring (3rd write queue)

_mult = mybir.AluOpType.mult
_add = mybir.AluOpType.add
_sub = mybir.AluOpType.subtract
_AF = mybir.ActivationFunctionType


def _tile_plan(nt: int):
    """B-tiles (ScalarE whole-tile stats): even t below nt-2.

    V-tiles (everything else) run on DVE.  agg_split: V-groups of tiles
    below this aggregate early (aggr1); the rest go through the short tail
    aggregation.
    """
    if nt < 8:
        return set(), 0
    b_set = {t for t in range(0, nt - 2, 2)}
    return b_set, nt - 5


def _make_body(f_half: int):
    nt = f_half // F_TILE
    last = nt - 1
    b_set, agg_split_tile = _tile_plan(nt)
    n_b = len(b_set)

    v_tiles = [t for t in range(nt) if t not in b_set]
    grp_of = {}
    g = 0
    ga = 0
    for t in v_tiles:
        grp_of[t] = g
        if t < agg_split_tile:
            ga = g + F_TILE // BN_F
        g += F_TILE // BN_F
    gtot = g
    # elems per partition behind aggr1 / tail-aggr (compile-time constants)
    n1 = sum(F_TILE for t in v_tiles if t < agg_split_tile)
    n2 = sum(F_TILE for t in v_tiles if t >= agg_split_tile)

    @with_exitstack
    def _body(ctx: ExitStack, tc: tile.TileContext,
              out_ap, xt_ap, gcol_ap, bcol_ap, fold1_ap, fold3_ap):
        nc = tc.nc
        ngrp = F_TILE // BN_F

        cache = ctx.enter_context(tc.tile_pool(name="cache", bufs=nt))
        lpool = ctx.enter_context(tc.tile_pool(name="lpool", bufs=LOAD_BUFS))
        opool = ctx.enter_context(tc.tile_pool(name="opool", bufs=OUT_BUFS))
        small = ctx.enter_context(tc.tile_pool(name="small", bufs=1))
        psum = ctx.enter_context(tc.tile_pool(name="psum", bufs=1, space="PSUM"))

        stats = small.tile([P, max(gtot, 1), 6], F32, tag="stats")
        accs = None
        psq = None
        if n_b:
            accs = small.tile([P, n_b, 2], F32, tag="accs")
            psq = psum.tile([P, F_TILE], F32, tag="psq")

        # Small inputs ride the SWDGE ring; they land well before first use
        # and never delay the HWDGE load burst.
        gcol_sb = small.tile([P, 1], F32, tag="gcol")
        bcol_sb = small.tile([P, 1], F32, tag="bcol")
        fold1_sb = small.tile([P, P], F32, tag="fold1")
        fold3_sb = small.tile([P, P], F32, tag="fold3")
        nc.gpsimd.dma_start(out=fold1_sb, in_=fold1_ap)
        nc.gpsimd.dma_start(out=fold3_sb, in_=fold3_ap)
        nc.gpsimd.dma_start(out=gcol_sb, in_=gcol_ap)
        nc.gpsimd.dma_start(out=bcol_sb, in_=bcol_ap)

        # Pre-load the ACT sqrt table so the tail doesn't pay ACT_TABLE_LOAD;
        # eps lives in a tiny tile (no const AP registered for 1e-5).
        warm = small.tile([P, 1], F32, tag="warm")
        nc.vector.memset(warm, 1.0)
        eps_sb = small.tile([P, 1], F32, tag="eps")
        nc.vector.memset(eps_sb, EPS)
        nc.scalar.activation(out=warm, in_=warm, func=_AF.Sqrt)

        ptot = psum.tile([P, 2], F32, tag="ptot")

        # ---- pass 1: single load stream on sync; B-tiles on ScalarE,
        # V-tiles on DVE ----
        cached = {}
        b_idx = {t: i for i, t in enumerate(sorted(b_set))}
        mva = small.tile([P, 2], F32, tag="mva")
        mvb = small.tile([P, 2], F32, tag="mvb")
        cols1 = small.tile([P, 2], F32, tag="cols1")
        mm1_emitted = False

        for t in range(nt):
            sl = slice(t * F_TILE, (t + 1) * F_TILE)
            xt16 = cache.tile([P, F_TILE], F16, tag="c")
            cached[t] = xt16
            xt32 = lpool.tile([P, F_TILE], F32, tag="l")
            if t == last:
                # Final tile in 4 chunks: DVE convert+stats pipeline with
                # the chunk DMAs; <1us of work follows the last landing.
                for j in range(ngrp):
                    cs = slice(t * F_TILE + j * BN_F, t * F_TILE + (j + 1) * BN_F)
                    nc.sync.dma_start(out=xt32[:, j * BN_F:(j + 1) * BN_F],
                                      in_=xt_ap[:, cs])
                for j in range(ngrp):
                    c32 = xt32[:, j * BN_F:(j + 1) * BN_F]
                    c16 = xt16[:, j * BN_F:(j + 1) * BN_F]
                    nc.vector.bn_stats(out=stats[:, grp_of[t] + j, :], in_=c32)
                    nc.vector.tensor_copy(out=c16, in_=c32)
            else:
                eng = nc.scalar if t in SCALAR_LOADS else nc.sync
                eng.dma_start(out=xt32, in_=xt_ap[:, sl])
                if t in b_set:
                    bi = b_idx[t]
                    nc.scalar.activation(out=xt16, in_=xt32, func=_AF.Copy,
                                         accum_out=accs[:, bi, 0:1])
                    # Square reads the fp16 copy: the fp32 lpool slot frees
                    # after the Copy alone (~2.3us instead of ~4.5us), so
                    # load pacing tracks the DMA, not the ACT pair.
                    nc.scalar.activation(out=psq, in_=xt16, func=_AF.Square,
                                         accum_out=accs[:, bi, 1:2])
                else:
                    nc.vector.tensor_copy(out=xt16, in_=xt32)
                    for j in range(ngrp):
                        nc.vector.bn_stats(
                            out=stats[:, grp_of[t] + j, :],
                            in_=xt32[:, j * BN_F:(j + 1) * BN_F])
            if t == agg_split_tile - 1 and ga > 0:
                # Early aggregation of V-groups so far -> first fold matmul
                # (runs on DVE/PE while the tail tiles stream in).
                nc.vector.bn_aggr(out=mva, in_=stats[:, :ga, :])
                nc.vector.tensor_mul(out=cols1[:, 1:2], in0=mva[:, 0:1],
                                     in1=mva[:, 0:1])
                nc.vector.tensor_add(out=cols1[:, 1:2], in0=cols1[:, 1:2],
                                     in1=mva[:, 1:2])
                nc.vector.tensor_scalar_mul(out=cols1[:, 1:2],
                                            in0=cols1[:, 1:2],
                                            scalar1=float(n1))
                nc.vector.tensor_scalar_mul(out=cols1[:, 0:1],
                                            in0=mva[:, 0:1],
                                            scalar1=float(n1))
                nc.tensor.matmul(out=ptot, lhsT=fold1_sb, rhs=cols1,
                                 start=True, stop=False)
                mm1_emitted = True

        # ---- tail aggregation: short bn_aggr + raw sums, merged with the
        # B accumulators into ONE final fold matmul ----
        asums = small.tile([P, 2], F32, tag="asums")
        if n_b:
            acc_view = accs.rearrange("p t c -> p c t")
            nc.vector.reduce_sum(out=asums, in_=acc_view,
                                 axis=mybir.AxisListType.X)
        else:
            nc.vector.memset(asums, 0.0)
        if ga > 0:
            nc.vector.bn_aggr(out=mvb, in_=stats[:, ga:, :])
        else:
            nc.vector.bn_aggr(out=mvb, in_=stats[:, :, :])
        cols2 = small.tile([P, 2], F32, tag="cols2")
        nc.vector.tensor_mul(out=cols2[:, 1:2], in0=mvb[:, 0:1], in1=mvb[:, 0:1])
        nc.vector.tensor_add(out=cols2[:, 1:2], in0=cols2[:, 1:2],
                             in1=mvb[:, 1:2])
        nc.vector.tensor_scalar_mul(out=cols2[:, 1:2], in0=cols2[:, 1:2],
                                    scalar1=float(n2 if ga > 0 else n1 + n2))
        nc.vector.tensor_scalar_mul(out=cols2[:, 0:1], in0=mvb[:, 0:1],
                                    scalar1=float(n2 if ga > 0 else n1 + n2))
        nc.vector.tensor_add(out=asums, in0=asums, in1=cols2)
        nc.tensor.matmul(out=ptot, lhsT=fold3_sb, rhs=asums,
                         start=not mm1_emitted, stop=True)

        # ---- per-channel coefficients ----
        tot = small.tile([P, 2], F32, tag="tot")   # (mean, E[x^2]) per channel
        nc.vector.tensor_copy(out=tot, in_=ptot)
        negv = small.tile([P, 1], F32, tag="negv")  # mean^2 - E[x^2]
        nc.vector.tensor_scalar(out=negv, in0=tot[:, 0:1],
                                scalar1=tot[:, 0:1], scalar2=tot[:, 1:2],
                                op0=_mult, op1=_sub)
        r = small.tile([P, 1], F32, tag="r")        # sqrt(var+eps) -> 1/...
        nc.scalar.activation(out=r, in_=negv, func=_AF.Sqrt,
                             scale=-1.0, bias=eps_sb[:, 0:1])
        nc.vector.reciprocal(out=r, in_=r)
        s_col = small.tile([P, 1], F32, tag="s_col")
        nc.vector.tensor_mul(out=s_col, in0=gcol_sb, in1=r)
        tneg = small.tile([P, 1], F32, tag="tneg")  # mean*s - beta
        nc.vector.tensor_scalar(out=tneg, in0=tot[:, 0:1],
                                scalar1=s_col[:, 0:1], scalar2=bcol_sb[:, 0:1],
                                op0=_mult, op1=_sub)

        # ---- pass 2: out32 = cache16*s - tneg; stores on all three rings.
        # Tile 0 goes out in halves so both HWDGE rings ramp ~0.8us sooner.
        half = F_TILE // 2
        o32 = opool.tile([P, F_TILE], F32, tag="o")
        for h in range(2):
            hs = slice(h * half, (h + 1) * half)
            nc.vector.tensor_scalar(out=o32[:, hs], in0=cached[0][:, hs],
                                    scalar1=s_col[:, 0:1], scalar2=tneg[:, 0:1],
                                    op0=_mult, op1=_sub)
            eng = nc.sync if h == 0 else nc.scalar
            eng.dma_start(out=out_ap[:, hs], in_=o32[:, hs])
        for i in range(1, nt):
            sl = slice(i * F_TILE, (i + 1) * F_TILE)
            o32 = opool.tile([P, F_TILE], F32, tag="o")
            nc.vector.tensor_scalar(out=o32, in0=cached[i],
                                    scalar1=s_col[:, 0:1], scalar2=tneg[:, 0:1],
                                    op0=_mult, op1=_sub)
            if i in SWDGE_STORES:
                nc.gpsimd.dma_start(out=out_ap[:, sl], in_=o32)
            elif i % 2 == 0:
                nc.sync.dma_start(out=out_ap[:, sl], in_=o32)
            else:
                nc.scalar.dma_start(out=out_ap[:, sl], in_=o32)

    return _body


_NC_CACHE = {}


def _build_program(f_half: int):
    if f_half in _NC_CACHE:
        return _NC_CACHE[f_half]
    nc = bacc.Bacc("TRN2", target_bir_lowering=False, debug=False,
                   num_devices=BATCH)
    xt = nc.dram_tensor("xt", [P, f_half], F32, kind="ExternalInput").ap()
    gcol = nc.dram_tensor("gcol", [P, 1], F32, kind="ExternalInput").ap()
    bcol = nc.dram_tensor("bcol", [P, 1], F32, kind="ExternalInput").ap()
    fold1 = nc.dram_tensor("fold1", [P, P], F32, kind="ExternalInput").ap()
    fold3 = nc.dram_tensor("fold3", [P, P], F32, kind="ExternalInput").ap()
    out = nc.dram_tensor("out", [P, f_half], F32, kind="ExternalOutput").ap()
    with tile.TileContext(nc) as tc:
        _make_body(f_half)(tc, out, xt, gcol, bcol, fold1, fold3)
    nc.compile()
    _NC_CACHE[f_half] = nc
    return nc


def _prepare(features, batch_indices, gamma, beta):
    features = np.asarray(features, dtype=np.float32)
    batch_indices = np.asarray(batch_indices, dtype=np.int32)
    gamma = np.asarray(gamma, dtype=np.float32)
    beta = np.asarray(beta, dtype=np.float32)

    bounds = np.searchsorted(batch_indices, np.arange(BATCH + 1), side="left")
    cnts = np.diff(bounds)
    # fixed SPMD shape: half-row length, padded to a multiple of F_TILE
    f_half = max(int(-(-int(cnts.max()) // 2 // F_TILE) * F_TILE), F_TILE)

    gcol = np.concatenate([gamma, gamma]).reshape(P, 1).astype(np.float32)
    bcol = np.concatenate([beta, beta]).reshape(P, 1).astype(np.float32)
    k = np.arange(P)
    foldm = (k[:, None] % C == k[None, :] % C).astype(np.float32)

    in_maps = []
    for b in range(BATCH):
        s, e = int(bounds[b]), int(bounds[b + 1])
        cnt = e - s
        xt = np.zeros((P, f_half), dtype=np.float32)
        npts1 = min(cnt, f_half)
        if npts1 > 0:
            xt[0:C, :npts1] = features[s : s + npts1].T
        if cnt > f_half:
            xt[C:P, : cnt - f_half] = features[s + f_half : e].T
        inv = 1.0 / max(cnt, 1)
        in_maps.append({
            "xt": xt,
            "gcol": gcol,
            "bcol": bcol,
            "fold1": (foldm * inv).astype(np.float32),
            "fold3": (foldm * inv).astype(np.float32),
        })
    return in_maps, bounds, f_half


def _assemble(results, bounds, f_half):
    out = np.empty((N, C), dtype=np.float32)
    for b in range(BATCH):
        s, e = int(bounds[b]), int(bounds[b + 1])
        cnt = e - s
        if cnt == 0:
            continue
        ot = results[b]["out"]
        npts1 = min(cnt, f_half)
        out[s : s + npts1] = ot[0:C, :npts1].T
        if cnt > f_half:
            out[s + f_half : e] = ot[C:P, : cnt - f_half].T
    return out


def run_with_results(features, batch_indices, gamma, beta, **run_kwargs):
    in_maps, bounds, f_half = _prepare(features, batch_indices, gamma, beta)
    nc = _build_program(f_half)
    res = run_bass_kernel_spmd(nc, in_maps, core_ids=list(range(BATCH)),
                               **run_kwargs)
    return _assemble(res.results, bounds, f_half), res


def kernel(features, batch_indices, gamma, beta):
    out, _ = run_with_results(features, batch_indices, gamma, beta)
    return out
